# revision 73
# baseline (speedup 1.0000x reference)
# Causal self-attention with RoPE, sharded over 8 TRN2 NeuronCores.
#
# Sharding: head-parallel. Each core owns 2 of the 16 heads (a 128-wide
# slice of the QKV projection output dims and of Wp's input dims) and
# computes a full [B*T, C] partial of the output projection. The host
# sums the 8 partials and adds bp.
#
# Device program (per core):
#   phase 1 (per 512-token group): q/k projections in natural [t, d] layout
#     (x-block stationary); v via weight-stationary vT matmuls (no rope on
#     v, so the transposed layout is free) then PE-transposed back. q/k are
#     evicted to SBUF bf16, rope applied on DVE along the free axis, then
#     PE-transposed to qT/kT [d=128, BT]. v staged as v_ext [tk, 65] with a
#     ones column so the PV matmul also produces the softmax denominator.
#   phase 2 (per batch, 512-wide q-span, BOTH heads interleaved — they are
#     independent chains in different PE row-groups): S^T blocks
#     [tk=128, tq<=512] on PE; exp on ScalarE writes P^T straight to SBUF;
#     causal zeroing of diagonal blocks via gpsimd affine_select (idle
#     engine); dense PV pass accumulates yT+l [65, 512] = v_ext.T @ P^T in
#     PSUM. Normalization: DVE reciprocal of the l row, PE rank-1 broadcast
#     matmul (f32), one DVE multiply writing yT_sb.
#   phase 3: out partial [t,e] tiles = yT.T @ WpT, f32, DMA'd out.
#   Emission interleaves group/attention/out-proj with one group of
#   lookahead so the in-order engine streams stay dense.
import math
from contextlib import ExitStack

import numpy as np
import ml_dtypes

import concourse.bass as bass
import concourse.mybir as mybir
import concourse.tile as tile
from concourse import bacc
from concourse.bass_utils import run_bass_kernel_spmd
from concourse.masks import make_identity

B, T, C, H = 2, 2048, 1024, 16
D = C // H          # 64, head dim
BT = B * T          # 4096 tokens
NCORES = 8
HPC = H // NCORES   # 2 heads per core
DPC = HPC * D       # 128 projection dims per core
NEG = -1e10
NT = BT // 128      # 32 token tiles
NTB = T // 128      # 16 token tiles per batch
NG = NT // 4        # 8 four-tile groups
NS = T // 512       # 4 q-spans per batch

F32 = mybir.dt.float32
F32R = mybir.dt.float32r
BF16 = mybir.dt.bfloat16


def _rope_cache_host():
    """Bit-exact replica of the reference's jax f32 rope cache, computed on
    the CPU backend (theta/cos/sin at large angles are sensitive to the
    exact f32 implementation, so this must go through jax, not numpy)."""
    import jax
    import jax.numpy as jnp

    cpu = jax.devices("cpu")[0]
    with jax.default_device(cpu):
        i = jnp.arange(D // 2, dtype=jnp.float32)
        theta = 1.0 / (10000.0 ** (-2.0 * (i - 1.0) / D))
        ang = jnp.arange(T, dtype=jnp.float32)[:, None] * theta[None, :]
        cos = np.asarray(jnp.cos(ang))
        sin = np.asarray(jnp.sin(ang))
    return cos, sin  # [T, D/2] f32


def _build_program():
    nc = bacc.Bacc("TRN2", target_bir_lowering=False, debug=False)

    xT = nc.dram_tensor("xT", [C, BT], BF16, kind="ExternalInput").ap()
    wqT = nc.dram_tensor("wqT", [C, DPC], BF16, kind="ExternalInput").ap()
    wkT = nc.dram_tensor("wkT", [C, DPC], BF16, kind="ExternalInput").ap()
    wvT = nc.dram_tensor("wvT", [C, DPC], BF16, kind="ExternalInput").ap()
    wpT = nc.dram_tensor("wpT", [DPC, C], BF16, kind="ExternalInput").ap()
    bq4 = nc.dram_tensor("bq4", [1, 512], BF16, kind="ExternalInput").ap()
    bk4 = nc.dram_tensor("bk4", [1, 512], BF16, kind="ExternalInput").ap()
    bv4 = nc.dram_tensor("bv4", [1, 512], BF16, kind="ExternalInput").ap()
    cosE = nc.dram_tensor("cosE", [T, D], BF16, kind="ExternalInput").ap()
    sinE = nc.dram_tensor("sinE", [T, D], BF16, kind="ExternalInput").ap()
    out = nc.dram_tensor("out_p", [BT, C], F32, kind="ExternalOutput").ap()

    with tile.TileContext(nc) as tc, ExitStack() as ctx:
        consts = ctx.enter_context(tc.tile_pool(name="consts", bufs=1))
        xpool = ctx.enter_context(tc.tile_pool(name="xpool", bufs=3))
        qkstage = ctx.enter_context(tc.tile_pool(name="qkstage", bufs=3))
        roptmp = ctx.enter_context(tc.tile_pool(name="roptmp", bufs=4))
        qkvn = ctx.enter_context(tc.tile_pool(name="qkvn", bufs=3))
        big = ctx.enter_context(tc.tile_pool(name="big", bufs=1))
        ppool = ctx.enter_context(tc.tile_pool(name="ppool", bufs=34))
        lpool = ctx.enter_context(tc.tile_pool(name="lpool", bufs=4))
        ostage = ctx.enter_context(tc.tile_pool(name="ostage", bufs=3))

        # PSUM banks (8 total): p1 pool shared by qkv-groups / transposes /
        # out-proj tiles (4), S^T + R tiles (2), yT+l accumulators (2).
        p1_ps = ctx.enter_context(tc.tile_pool(name="p1_ps", bufs=3, space="PSUM"))
        s_ps = ctx.enter_context(tc.tile_pool(name="s_ps", bufs=3, space="PSUM"))
        ytl_ps = ctx.enter_context(tc.tile_pool(name="ytl_ps", bufs=2, space="PSUM"))

        # ---- constants ----
        ident = consts.tile([128, 128], BF16)
        make_identity(nc, ident)
        ones64f = consts.tile([1, D], F32)
        nc.vector.memset(ones64f, 1.0)
        ones_row = consts.tile([1, 128], BF16)
        nc.vector.memset(ones_row, 1.0)
        ones512 = consts.tile([1, 512], BF16)
        nc.vector.memset(ones512, 1.0)

        w_sb = {}
        b_sb = {}
        for name, wt, bt in (("q", wqT, bq4), ("k", wkT, bk4), ("v", wvT, bv4)):
            w = consts.tile([128, 8, DPC], BF16, name=f"w{name}_sb")
            nc.sync.dma_start(out=w, in_=wt.rearrange("(k p) d -> p k d", p=128))
            w_sb[name] = w
            b = consts.tile([1, 512], BF16, name=f"b{name}_sb")
            nc.sync.dma_start(out=b, in_=bt)
            b_sb[name] = b
        cos_sb = consts.tile([128, NTB, D], BF16)
        sin_sb = consts.tile([128, NTB, D], BF16)
        wp_sb = consts.tile([128, C], BF16)

        def emit_late_consts():
            # wp is first read by the out-projection of span 0, long after
            # startup — don't let its 256KB delay the first x chunks
            nc.sync.dma_start(out=wp_sb, in_=wpT)

        # persistent activations
        qT_sb = big.tile([128, BT], BF16)   # rows: [h0 d0..63, h1 d0..63]
        kT_sb = big.tile([128, BT], BF16)
        vext_sb = big.tile([128, NT, HPC, D + 1], BF16)  # [tk, tile, head, d+1]
        yT_sb = big.tile([128, BT], BF16)

        nc.vector.memset(vext_sb[:, :, :, D:D + 1], 1.0)  # ones column

        xT_g = xT.rearrange("(k p) (g q) -> g p k q", p=128, q=512)

        # ---- phase 1 (emitted per 512-token group): QKV + rope + transposes
        def load_x(g, name=None):
            x_t = xpool.tile([128, 8, 512], BF16, tag="x_t",
                             name=name or f"x_t_{g}")
            # one DMA per contraction tile: spreads the 1MB load across
            # HWDGE queues and lets the kk=0 matmuls start early
            for kk in range(8):
                nc.sync.dma_start(out=x_t[:, kk, :], in_=xT_g[g, :, kk, :])
            return x_t

        def emit_group(g, x_t=None):
            if x_t is None:
                x_t = load_x(g)
            gtb = (g * 4) % NTB  # first in-batch token tile of the group
            ps = {}
            for name in ("q", "k"):
                ps[name] = p1_ps.tile([128, 512], F32, tag="p512",
                                      name=f"ps_{name}_{g}")
                # bias via rank-1 matmul opening the group over the whole tile
                nc.tensor.matmul(
                    ps[name], lhsT=ones_row, rhs=b_sb[name],
                    start=True, stop=False,
                )
            # v has no rope: weight-stationary vT = WvT.T @ xT (N=512),
            # 8 matmuls instead of 64, then PE-transpose back to natural.
            psvT = p1_ps.tile([128, 512], F32, tag="p512", name=f"ps_vT_{g}")
            nc.tensor.matmul(
                psvT, lhsT=b_sb["v"][:, 0:128], rhs=ones512,
                start=True, stop=False,
            )
            for kk in range(8):
                nc.tensor.matmul(
                    psvT, lhsT=w_sb["v"][:, kk, :], rhs=x_t[:, kk, :],
                    start=False, stop=(kk == 7),
                )
            for name in ("q", "k"):
                for n in range(4):
                    for kk in range(8):
                        nc.tensor.matmul(
                            ps[name][:, n * 128:(n + 1) * 128],
                            lhsT=x_t[:, kk, n * 128:(n + 1) * 128],
                            rhs=w_sb[name][:, kk, :],
                            start=False, stop=(n == 3 and kk == 7),
                        )
            # vT -> bf16 stage -> transpose -> v_ext staging (per head)
            vstg = qkstage.tile([128, 512], BF16, tag="stg", name=f"vstg_{g}")
            nc.scalar.copy(out=vstg, in_=psvT)
            tpv = p1_ps.tile([128, 512], BF16, tag="p512", name=f"tpv_{g}")
            for n in range(4):
                nc.tensor.transpose(
                    tpv[:, n * 128:(n + 1) * 128],
                    vstg[:, n * 128:(n + 1) * 128], ident,
                )
            tpv4 = tpv.rearrange("p (n hh d) -> p n hh d", hh=HPC, d=D)
            for h in range(HPC):
                nc.vector.tensor_copy(
                    out=vext_sb[:, g * 4:(g + 1) * 4, h, 0:D],
                    in_=tpv4[:, :, h, :],
                )
            # rope on q, k: evict to SBUF bf16 then all-SBUF DVE (2x mode)
            ct = cos_sb[:, gtb:gtb + 4, :]
            st = sin_sb[:, gtb:gtb + 4, :]
            for name in ("q", "k"):
                stg = qkstage.tile([128, 512], BF16, tag="stg",
                                   name=f"stg_{name}_{g}")
                nc.scalar.copy(out=stg, in_=ps[name])
                s4 = stg.rearrange("p (n d2) -> p n d2", n=4)
                ev, od = s4[:, :, 0:DPC:2], s4[:, :, 1:DPC:2]
                t1 = roptmp.tile([128, 4, D], BF16, tag="t1")
                t2 = roptmp.tile([128, 4, D], BF16, tag="t2")
                nc.vector.tensor_mul(t1, ev, ct)
                nc.vector.tensor_mul(t2, od, st)
                qn = qkvn.tile([128, 512], BF16, tag="qn", name=f"{name}n_{g}")
                qn4 = qn.rearrange("p (n d2) -> p n d2", n=4)
                nc.vector.tensor_sub(qn4[:, :, 0:DPC:2], t1, t2)
                t3 = roptmp.tile([128, 4, D], BF16, tag="t3")
                t4 = roptmp.tile([128, 4, D], BF16, tag="t4")
                nc.gpsimd.tensor_mul(t3, ev, st)
                nc.gpsimd.tensor_mul(t4, od, ct)
                nc.vector.tensor_add(qn4[:, :, 1:DPC:2], t3, t4)
                tp = p1_ps.tile([128, 512], BF16, tag="p512",
                                name=f"tp_{name}_{g}")
                for n in range(4):
                    nc.tensor.transpose(
                        tp[:, n * 128:(n + 1) * 128],
                        qn[:, n * 128:(n + 1) * 128], ident,
                    )
                dst = qT_sb if name == "q" else kT_sb
                nc.vector.tensor_copy(out=dst[:, g * 512:(g + 1) * 512], in_=tp)

        # ---- phase 2: attention for one (batch, q-span), both heads
        # interleaved. The two heads are independent S->exp->PV chains and
        # their S matmuls sit in different PE row-groups (partition 0 / 64),
        # so interleaving keeps both PE and ScalarE dense.
        def emit_attention(b, s):
            rows = {h: slice(h * D, (h + 1) * D) for h in range(HPC)}
            ytl = {
                h: ytl_ps.tile([D + 1, 512], F32, tag="ytl",
                               name=f"ytl_{b}_{h}_{s}")
                for h in range(HPC)
            }
            nj = 4 * s + 4
            pts = {h: [] for h in range(HPC)}
            for j in range(nj):
                dj = j - 4 * s
                coff = max(dj, 0) * 128
                n0 = 512 - coff
                for h in range(HPC):
                    sp = s_ps.tile([128, 512], F32, tag="sp",
                                   name=f"sp_{b}_{h}_{s}_{j}")
                    nc.tensor.matmul(
                        sp[:, :n0],
                        lhsT=kT_sb[rows[h], b * T + j * 128:b * T + (j + 1) * 128],
                        rhs=qT_sb[rows[h], b * T + s * 512 + coff:b * T + (s + 1) * 512],
                        start=True, stop=True,
                    )
                    pt = ppool.tile([128, 512], BF16, tag="pt",
                                    name=f"pt_{b}_{h}_{s}_{j}")
                    nc.scalar.activation(
                        out=pt[:, :n0], in_=sp[:, :n0],
                        func=mybir.ActivationFunctionType.Exp,
                    )
                    if dj >= 0:
                        # causal zeroing of the diagonal 128-block:
                        # keep where tk <= tq, i.e. (tq - tk) >= 0
                        nc.gpsimd.affine_select(
                            out=pt[:, 0:128], in_=pt[:, 0:128],
                            compare_op=mybir.AluOpType.is_ge,
                            fill=0.0, base=0,
                            pattern=[[1, 128]], channel_multiplier=-1,
                        )
                    pts[h].append((pt, coff, n0))
            for j in range(nj):
                for h in range(HPC):
                    pt, coff, n0 = pts[h][j]
                    nc.tensor.matmul(
                        ytl[h][:, coff:512],
                        lhsT=vext_sb[:, b * NTB + j, h, :],
                        rhs=pt[:, :n0],
                        start=(j == 0), stop=(j == nj - 1),
                    )
            for h in range(HPC):
                # normalize: r = 1/l; PE rank-1 broadcast; one multiply
                rcp = lpool.tile([1, 512], F32, tag="rcp",
                                 name=f"rcp_{b}_{h}_{s}")
                nc.vector.reciprocal(rcp, ytl[h][D:D + 1, :])
                rps = s_ps.tile([D, 512], F32, tag="sp", name=f"rps_{b}_{h}_{s}")
                nc.tensor.matmul(
                    rps, lhsT=ones64f, rhs=rcp,
                    start=True, stop=True,
                )
                rbc = lpool.tile([D, 512], F32, tag="rbc",
                                 name=f"rbc_{b}_{h}_{s}")
                nc.vector.tensor_copy(out=rbc, in_=rps)
                nc.vector.tensor_mul(
                    yT_sb[rows[h], b * T + s * 512:b * T + (s + 1) * 512],
                    ytl[h][0:D, :], rbc,
                )

        # ---- phase 3: output projection (partial) for one token tile ----
        def emit_out(i):
            ob = ostage.tile([128, C], F32, tag="ob", name=f"ob_{i}")
            for e in range(2):
                op = p1_ps.tile([128, 512], F32, tag="p512", name=f"op_{i}_{e}")
                nc.tensor.matmul(
                    op, lhsT=yT_sb[:, i * 128:(i + 1) * 128],
                    rhs=wp_sb[:, e * 512:(e + 1) * 512], start=True, stop=True,
                )
                if e == 0:
                    nc.vector.tensor_copy(out=ob[:, 0:512], in_=op)
                else:
                    nc.scalar.copy(out=ob[:, 512:1024], in_=op)
            nc.sync.dma_start(out=out[i * 128:(i + 1) * 128, :], in_=ob)

        # ---- interleaved emission, one group of lookahead so PE has
        # independent projection work queued while attention waits on
        # rope/transpose round-trips.
        seq = [(b, s) for b in range(B) for s in range(NS)]
        x0 = load_x(0, name="x_t_pre0")
        # tables land while group 0's projection matmuls run; they are
        # only read by the rope, several microseconds in
        nc.sync.dma_start(out=cos_sb,
                          in_=cosE.rearrange("(n p) d -> p n d", p=128))
        nc.sync.dma_start(out=sin_sb,
                          in_=sinE.rearrange("(n p) d -> p n d", p=128))
        emit_group(0, x_t=x0)
        emit_late_consts()
        for idx, (b, s) in enumerate(seq):
            if idx + 1 < len(seq):
                emit_group(idx + 1)
            emit_attention(b, s)
            for n in range(4):
                emit_out(b * NTB + s * 4 + n)

    nc.compile()
    return nc


_nc_cache = None


def _get_program():
    global _nc_cache
    if _nc_cache is None:
        _nc_cache = _build_program()
    return _nc_cache


def _host_inputs(x, Wq, bq, Wk, bk, Wv, bv, Wp, bp):
    bf = ml_dtypes.bfloat16
    scale = 1.0 / math.sqrt(D)
    x2 = np.ascontiguousarray(np.asarray(x, np.float32).reshape(BT, C).T)  # [C, BT]
    xT_b = x2.astype(bf)
    cos, sin = _rope_cache_host()  # [T, D/2]
    cosE = np.concatenate([cos, cos], axis=1)  # [T, D] (2 heads' even cols)
    sinE = np.concatenate([sin, sin], axis=1)

    common = {
        "xT": xT_b,
        "cosE": np.ascontiguousarray(cosE).astype(bf),
        "sinE": np.ascontiguousarray(sinE).astype(bf),
    }
    in_maps = []
    for m in range(NCORES):
        sl = slice(m * DPC, (m + 1) * DPC)
        in_maps.append({
            **common,
            "wqT": np.ascontiguousarray((np.asarray(Wq, np.float32)[sl] * scale).T).astype(bf),
            "wkT": np.ascontiguousarray(np.asarray(Wk, np.float32)[sl].T).astype(bf),
            "wvT": np.ascontiguousarray(np.asarray(Wv, np.float32)[sl].T).astype(bf),
            "wpT": np.ascontiguousarray(np.asarray(Wp, np.float32)[:, sl].T).astype(bf),
            "bq4": np.tile((np.asarray(bq, np.float32)[sl] * scale), 4).reshape(1, 512).astype(bf),
            "bk4": np.tile(np.asarray(bk, np.float32)[sl], 4).reshape(1, 512).astype(bf),
            "bv4": np.tile(np.asarray(bv, np.float32)[sl], 4).reshape(1, 512).astype(bf),
        })
    return in_maps


def kernel(x, Wq, bq, Wk, bk, Wv, bv, Wp, bp, _run_kwargs=None):
    nc = _get_program()
    in_maps = _host_inputs(x, Wq, bq, Wk, bk, Wv, bv, Wp, bp)
    res = run_bass_kernel_spmd(
        nc, in_maps, core_ids=list(range(NCORES)), **(_run_kwargs or {})
    )
    partials = [r["out_p"] for r in res.results]
    acc = np.zeros((BT, C), np.float64)
    for p in partials:
        acc += p
    out = acc.astype(np.float32) + np.asarray(bp, np.float32)[None, :]
    if _run_kwargs:
        kernel.last_results = res
    return out.reshape(B, T, C)


# revision 82
# speedup vs baseline: 1.0542x; 1.0542x over previous
# Causal self-attention with RoPE, sharded over 8 TRN2 NeuronCores.
#
# Sharding: head-parallel. Each core owns 2 of the 16 heads (a 128-wide
# slice of the QKV projection output dims and of Wp's input dims) and
# computes a full [B*T, C] partial of the output projection. The host
# sums the 8 partials and adds bp.
#
# Device program (per core):
#   phase 1 (per 512-token group): q/k projections in natural [t, d] layout
#     (x-block stationary); v via weight-stationary vT matmuls (no rope on
#     v, so the transposed layout is free) then PE-transposed back. q/k are
#     evicted to SBUF bf16, rope applied on DVE along the free axis, then
#     PE-transposed to qT/kT [d=128, BT]. v staged as v_ext [tk, 65] with a
#     ones column so the PV matmul also produces the softmax denominator.
#   phase 2 (per batch, 512-wide q-span, BOTH heads interleaved — they are
#     independent chains in different PE row-groups): S^T blocks
#     [tk=128, tq<=512] on PE; exp on ScalarE writes P^T straight to SBUF;
#     causal zeroing of diagonal blocks via gpsimd affine_select (idle
#     engine); dense PV pass accumulates yT+l [65, 512] = v_ext.T @ P^T in
#     PSUM. Normalization: DVE reciprocal of the l row, PE rank-1 broadcast
#     matmul (f32), one DVE multiply writing yT_sb.
#   phase 3: out partial [t,e] tiles = yT.T @ WpT, f32, DMA'd out.
#   Emission interleaves group/attention/out-proj with one group of
#   lookahead so the in-order engine streams stay dense.
import math
from contextlib import ExitStack

import numpy as np
import ml_dtypes

import concourse.bass as bass
import concourse.mybir as mybir
import concourse.tile as tile
from concourse import bacc
from concourse.bass_utils import run_bass_kernel_spmd
from concourse.masks import make_identity

B, T, C, H = 2, 2048, 1024, 16
D = C // H          # 64, head dim
BT = B * T          # 4096 tokens
NCORES = 8
HPC = H // NCORES   # 2 heads per core
DPC = HPC * D       # 128 projection dims per core
NEG = -1e10
NT = BT // 128      # 32 token tiles
NTB = T // 128      # 16 token tiles per batch
NG = NT // 4        # 8 four-tile groups
NS = T // 512       # 4 q-spans per batch

F32 = mybir.dt.float32
F32R = mybir.dt.float32r
BF16 = mybir.dt.bfloat16


def _rope_cache_host():
    """Bit-exact replica of the reference's jax f32 rope cache, computed on
    the CPU backend (theta/cos/sin at large angles are sensitive to the
    exact f32 implementation, so this must go through jax, not numpy)."""
    import jax
    import jax.numpy as jnp

    cpu = jax.devices("cpu")[0]
    with jax.default_device(cpu):
        i = jnp.arange(D // 2, dtype=jnp.float32)
        theta = 1.0 / (10000.0 ** (-2.0 * (i - 1.0) / D))
        ang = jnp.arange(T, dtype=jnp.float32)[:, None] * theta[None, :]
        cos = np.asarray(jnp.cos(ang))
        sin = np.asarray(jnp.sin(ang))
    return cos, sin  # [T, D/2] f32


def _build_program():
    nc = bacc.Bacc("TRN2", target_bir_lowering=False, debug=False)

    xT = nc.dram_tensor("xT", [C, BT], BF16, kind="ExternalInput").ap()
    wqT = nc.dram_tensor("wqT", [C, DPC], BF16, kind="ExternalInput").ap()
    wkT = nc.dram_tensor("wkT", [C, DPC], BF16, kind="ExternalInput").ap()
    wvT = nc.dram_tensor("wvT", [C, DPC], BF16, kind="ExternalInput").ap()
    wpT = nc.dram_tensor("wpT", [DPC, C], BF16, kind="ExternalInput").ap()
    bq4 = nc.dram_tensor("bq4", [1, 512], BF16, kind="ExternalInput").ap()
    bk4 = nc.dram_tensor("bk4", [1, 512], BF16, kind="ExternalInput").ap()
    bv4 = nc.dram_tensor("bv4", [1, 512], BF16, kind="ExternalInput").ap()
    cosE = nc.dram_tensor("cosE", [T, D], BF16, kind="ExternalInput").ap()
    sinE = nc.dram_tensor("sinE", [T, D], BF16, kind="ExternalInput").ap()
    out = nc.dram_tensor("out_p", [BT, C], F32, kind="ExternalOutput").ap()

    with tile.TileContext(nc) as tc, ExitStack() as ctx:
        consts = ctx.enter_context(tc.tile_pool(name="consts", bufs=1))
        xpool = ctx.enter_context(tc.tile_pool(name="xpool", bufs=3))
        qkstage = ctx.enter_context(tc.tile_pool(name="qkstage", bufs=4))
        roptmp = ctx.enter_context(tc.tile_pool(name="roptmp", bufs=4))
        qkvn = ctx.enter_context(tc.tile_pool(name="qkvn", bufs=3))
        big = ctx.enter_context(tc.tile_pool(name="big", bufs=1))
        ppool = ctx.enter_context(tc.tile_pool(name="ppool", bufs=34))
        lpool = ctx.enter_context(tc.tile_pool(name="lpool", bufs=4))
        ostage = ctx.enter_context(tc.tile_pool(name="ostage", bufs=6))

        # PSUM banks (8 total): p1 pool shared by qkv-groups / transposes /
        # out-proj tiles (4), S^T + R tiles (2), yT+l accumulators (2).
        p1_ps = ctx.enter_context(tc.tile_pool(name="p1_ps", bufs=3, space="PSUM"))
        s_ps = ctx.enter_context(tc.tile_pool(name="s_ps", bufs=3, space="PSUM"))
        ytl_ps = ctx.enter_context(tc.tile_pool(name="ytl_ps", bufs=2, space="PSUM"))

        # ---- constants ----
        ident = consts.tile([128, 128], BF16)
        make_identity(nc, ident)
        ones64f = consts.tile([1, D], F32)
        nc.vector.memset(ones64f, 1.0)
        ones_row = consts.tile([1, 128], BF16)
        nc.vector.memset(ones_row, 1.0)
        ones512 = consts.tile([1, 512], BF16)
        nc.vector.memset(ones512, 1.0)

        w_sb = {}
        b_sb = {}
        for name, wt, bt in (("q", wqT, bq4), ("k", wkT, bk4), ("v", wvT, bv4)):
            w = consts.tile([128, 8, DPC], BF16, name=f"w{name}_sb")
            nc.sync.dma_start(out=w, in_=wt.rearrange("(k p) d -> p k d", p=128))
            w_sb[name] = w
            b = consts.tile([1, 512], BF16, name=f"b{name}_sb")
            nc.sync.dma_start(out=b, in_=bt)
            b_sb[name] = b
        cos_sb = consts.tile([128, NTB, D], BF16)
        sin_sb = consts.tile([128, NTB, D], BF16)
        wp_sb = consts.tile([128, C], BF16)

        def emit_late_consts():
            # wp is first read by the out-projection of span 0, long after
            # startup — don't let its 256KB delay the first x chunks
            nc.sync.dma_start(out=wp_sb, in_=wpT)

        # persistent activations
        qT_sb = big.tile([128, BT], BF16)   # rows: [h0 d0..63, h1 d0..63]
        kT_sb = big.tile([128, BT], BF16)
        vext_sb = big.tile([128, NT, HPC, D + 1], BF16)  # [tk, tile, head, d+1]
        yT_sb = big.tile([128, BT], BF16)

        nc.vector.memset(vext_sb[:, :, :, D:D + 1], 1.0)  # ones column

        xT_g = xT.rearrange("(k p) (g q) -> g p k q", p=128, q=512)

        # ---- phase 1 (emitted per 512-token group): QKV + rope + transposes
        def load_x(g, name=None):
            x_t = xpool.tile([128, 8, 512], BF16, tag="x_t",
                             name=name or f"x_t_{g}")
            # one DMA per contraction tile: spreads the 1MB load across
            # HWDGE queues and lets the kk=0 matmuls start early
            for kk in range(8):
                nc.sync.dma_start(out=x_t[:, kk, :], in_=xT_g[g, :, kk, :])
            return x_t

        def emit_group(g, x_t=None):
            if x_t is None:
                x_t = load_x(g)
            gtb = (g * 4) % NTB  # first in-batch token tile of the group
            ps = {}
            for name in ("q", "k"):
                ps[name] = p1_ps.tile([128, 512], F32, tag="p512",
                                      name=f"ps_{name}_{g}")
                # bias via rank-1 matmul opening the group over the whole tile
                nc.tensor.matmul(
                    ps[name], lhsT=ones_row, rhs=b_sb[name],
                    start=True, stop=False,
                )
            # v has no rope: weight-stationary vT = WvT.T @ xT (N=512),
            # 8 matmuls instead of 64, then PE-transpose back to natural.
            psvT = p1_ps.tile([128, 512], F32, tag="p512", name=f"ps_vT_{g}")
            nc.tensor.matmul(
                psvT, lhsT=b_sb["v"][:, 0:128], rhs=ones512,
                start=True, stop=False,
            )
            for kk in range(8):
                nc.tensor.matmul(
                    psvT, lhsT=w_sb["v"][:, kk, :], rhs=x_t[:, kk, :],
                    start=False, stop=(kk == 7),
                )
            for name in ("q", "k"):
                for n in range(4):
                    for kk in range(8):
                        nc.tensor.matmul(
                            ps[name][:, n * 128:(n + 1) * 128],
                            lhsT=x_t[:, kk, n * 128:(n + 1) * 128],
                            rhs=w_sb[name][:, kk, :],
                            start=False, stop=(n == 3 and kk == 7),
                        )
            # vT -> bf16 stage -> transpose -> v_ext staging (per head)
            vstg = qkstage.tile([128, 512], BF16, tag="stg", name=f"vstg_{g}")
            nc.scalar.copy(out=vstg, in_=psvT)
            tpv = p1_ps.tile([128, 512], BF16, tag="p512", name=f"tpv_{g}")
            for n in range(4):
                nc.tensor.transpose(
                    tpv[:, n * 128:(n + 1) * 128],
                    vstg[:, n * 128:(n + 1) * 128], ident,
                )
            tpv4 = tpv.rearrange("p (n hh d) -> p n hh d", hh=HPC, d=D)
            for h in range(HPC):
                nc.vector.tensor_copy(
                    out=vext_sb[:, g * 4:(g + 1) * 4, h, 0:D],
                    in_=tpv4[:, :, h, :],
                )
            # rope on q, k: evict to SBUF bf16 then all-SBUF DVE (2x mode)
            ct = cos_sb[:, gtb:gtb + 4, :]
            st = sin_sb[:, gtb:gtb + 4, :]
            for name in ("q", "k"):
                stg = qkstage.tile([128, 512], BF16, tag="stg",
                                   name=f"stg_{name}_{g}")
                nc.scalar.copy(out=stg, in_=ps[name])
                s4 = stg.rearrange("p (n d2) -> p n d2", n=4)
                ev, od = s4[:, :, 0:DPC:2], s4[:, :, 1:DPC:2]
                t1 = roptmp.tile([128, 4, D], BF16, tag="t1")
                t2 = roptmp.tile([128, 4, D], BF16, tag="t2")
                nc.vector.tensor_mul(t1, ev, ct)
                nc.vector.tensor_mul(t2, od, st)
                qn = qkvn.tile([128, 512], BF16, tag="qn", name=f"{name}n_{g}")
                qn4 = qn.rearrange("p (n d2) -> p n d2", n=4)
                nc.vector.tensor_sub(qn4[:, :, 0:DPC:2], t1, t2)
                t3 = roptmp.tile([128, 4, D], BF16, tag="t3")
                t4 = roptmp.tile([128, 4, D], BF16, tag="t4")
                nc.gpsimd.tensor_mul(t3, ev, st)
                nc.gpsimd.tensor_mul(t4, od, ct)
                nc.vector.tensor_add(qn4[:, :, 1:DPC:2], t3, t4)
                tp = p1_ps.tile([128, 512], BF16, tag="p512",
                                name=f"tp_{name}_{g}")
                for n in range(4):
                    nc.tensor.transpose(
                        tp[:, n * 128:(n + 1) * 128],
                        qn[:, n * 128:(n + 1) * 128], ident,
                    )
                dst = qT_sb if name == "q" else kT_sb
                nc.vector.tensor_copy(out=dst[:, g * 512:(g + 1) * 512], in_=tp)

        # ---- phase 2: attention for one (batch, q-span), both heads
        # interleaved. The two heads are independent S->exp->PV chains and
        # their S matmuls sit in different PE row-groups (partition 0 / 64),
        # so interleaving keeps both PE and ScalarE dense.
        def emit_attention(b, s):
            rows = {h: slice(h * D, (h + 1) * D) for h in range(HPC)}
            ytl = {
                h: ytl_ps.tile([D + 1, 512], F32, tag="ytl",
                               name=f"ytl_{b}_{h}_{s}")
                for h in range(HPC)
            }
            nj = 4 * s + 4
            pts = {h: [] for h in range(HPC)}
            for j in range(nj):
                dj = j - 4 * s
                coff = max(dj, 0) * 128
                n0 = 512 - coff
                for h in range(HPC):
                    sp = s_ps.tile([128, 512], F32, tag="sp",
                                   name=f"sp_{b}_{h}_{s}_{j}")
                    nc.tensor.matmul(
                        sp[:, :n0],
                        lhsT=kT_sb[rows[h], b * T + j * 128:b * T + (j + 1) * 128],
                        rhs=qT_sb[rows[h], b * T + s * 512 + coff:b * T + (s + 1) * 512],
                        start=True, stop=True,
                    )
                    pt = ppool.tile([128, 512], BF16, tag="pt",
                                    name=f"pt_{b}_{h}_{s}_{j}")
                    nc.scalar.activation(
                        out=pt[:, :n0], in_=sp[:, :n0],
                        func=mybir.ActivationFunctionType.Exp,
                    )
                    if dj >= 0:
                        # causal zeroing of the diagonal 128-block:
                        # keep where tk <= tq, i.e. (tq - tk) >= 0
                        nc.gpsimd.affine_select(
                            out=pt[:, 0:128], in_=pt[:, 0:128],
                            compare_op=mybir.AluOpType.is_ge,
                            fill=0.0, base=0,
                            pattern=[[1, 128]], channel_multiplier=-1,
                        )
                    pts[h].append((pt, coff, n0))
            for j in range(nj):
                for h in range(HPC):
                    pt, coff, n0 = pts[h][j]
                    nc.tensor.matmul(
                        ytl[h][:, coff:512],
                        lhsT=vext_sb[:, b * NTB + j, h, :],
                        rhs=pt[:, :n0],
                        start=(j == 0), stop=(j == nj - 1),
                    )
            for h in range(HPC):
                # normalize: r = 1/l; PE rank-1 broadcast; one multiply
                rcp = lpool.tile([1, 512], F32, tag="rcp",
                                 name=f"rcp_{b}_{h}_{s}")
                nc.vector.reciprocal(rcp, ytl[h][D:D + 1, :])
                rps = s_ps.tile([D, 512], F32, tag="sp", name=f"rps_{b}_{h}_{s}")
                nc.tensor.matmul(
                    rps, lhsT=ones64f, rhs=rcp,
                    start=True, stop=True,
                )
                rbc = lpool.tile([D, 512], F32, tag="rbc",
                                 name=f"rbc_{b}_{h}_{s}")
                nc.vector.tensor_copy(out=rbc, in_=rps)
                nc.vector.tensor_mul(
                    yT_sb[rows[h], b * T + s * 512:b * T + (s + 1) * 512],
                    ytl[h][0:D, :], rbc,
                )

        # ---- phase 3: output projection (partial) for one token tile ----
        def emit_out(i):
            ob = ostage.tile([128, C], F32, tag="ob", name=f"ob_{i}")
            for e in range(2):
                op = ytl_ps.tile([128, 512], F32, tag="ytl", name=f"op_{i}_{e}")
                nc.tensor.matmul(
                    op, lhsT=yT_sb[:, i * 128:(i + 1) * 128],
                    rhs=wp_sb[:, e * 512:(e + 1) * 512], start=True, stop=True,
                )
                if e == 0:
                    nc.vector.tensor_copy(out=ob[:, 0:512], in_=op)
                else:
                    nc.scalar.copy(out=ob[:, 512:1024], in_=op)
                nc.sync.dma_start(
                    out=out[i * 128:(i + 1) * 128, e * 512:(e + 1) * 512],
                    in_=ob[:, e * 512:(e + 1) * 512],
                )

        # ---- interleaved emission, one group of lookahead so PE has
        # independent projection work queued while attention waits on
        # rope/transpose round-trips.
        seq = [(b, s) for b in range(B) for s in range(NS)]
        x0 = load_x(0, name="x_t_pre0")
        # tables land while group 0's projection matmuls run; they are
        # only read by the rope, several microseconds in
        nc.sync.dma_start(out=cos_sb,
                          in_=cosE.rearrange("(n p) d -> p n d", p=128))
        nc.sync.dma_start(out=sin_sb,
                          in_=sinE.rearrange("(n p) d -> p n d", p=128))
        emit_group(0, x_t=x0)
        emit_late_consts()
        for idx, (b, s) in enumerate(seq):
            if idx + 1 < len(seq):
                emit_group(idx + 1)
            emit_attention(b, s)
            for n in range(4):
                emit_out(b * NTB + s * 4 + n)

    nc.compile()
    return nc


_nc_cache = None


def _get_program():
    global _nc_cache
    if _nc_cache is None:
        _nc_cache = _build_program()
    return _nc_cache


def _host_inputs(x, Wq, bq, Wk, bk, Wv, bv, Wp, bp):
    bf = ml_dtypes.bfloat16
    scale = 1.0 / math.sqrt(D)
    x2 = np.ascontiguousarray(np.asarray(x, np.float32).reshape(BT, C).T)  # [C, BT]
    xT_b = x2.astype(bf)
    cos, sin = _rope_cache_host()  # [T, D/2]
    cosE = np.concatenate([cos, cos], axis=1)  # [T, D] (2 heads' even cols)
    sinE = np.concatenate([sin, sin], axis=1)

    common = {
        "xT": xT_b,
        "cosE": np.ascontiguousarray(cosE).astype(bf),
        "sinE": np.ascontiguousarray(sinE).astype(bf),
    }
    in_maps = []
    for m in range(NCORES):
        sl = slice(m * DPC, (m + 1) * DPC)
        in_maps.append({
            **common,
            "wqT": np.ascontiguousarray((np.asarray(Wq, np.float32)[sl] * scale).T).astype(bf),
            "wkT": np.ascontiguousarray(np.asarray(Wk, np.float32)[sl].T).astype(bf),
            "wvT": np.ascontiguousarray(np.asarray(Wv, np.float32)[sl].T).astype(bf),
            "wpT": np.ascontiguousarray(np.asarray(Wp, np.float32)[:, sl].T).astype(bf),
            "bq4": np.tile((np.asarray(bq, np.float32)[sl] * scale), 4).reshape(1, 512).astype(bf),
            "bk4": np.tile(np.asarray(bk, np.float32)[sl], 4).reshape(1, 512).astype(bf),
            "bv4": np.tile(np.asarray(bv, np.float32)[sl], 4).reshape(1, 512).astype(bf),
        })
    return in_maps


def kernel(x, Wq, bq, Wk, bk, Wv, bv, Wp, bp, _run_kwargs=None):
    nc = _get_program()
    in_maps = _host_inputs(x, Wq, bq, Wk, bk, Wv, bv, Wp, bp)
    res = run_bass_kernel_spmd(
        nc, in_maps, core_ids=list(range(NCORES)), **(_run_kwargs or {})
    )
    partials = [r["out_p"] for r in res.results]
    acc = np.zeros((BT, C), np.float64)
    for p in partials:
        acc += p
    out = acc.astype(np.float32) + np.asarray(bp, np.float32)[None, :]
    if _run_kwargs:
        kernel.last_results = res
    return out.reshape(B, T, C)
